# revision 20
# baseline (speedup 1.0000x reference)
"""Single-head attention (SEQ=8192, EMBED=2048, HEAD=128) on 8 TRN2 NeuronCores.

Sharding: queries (rows of Q / the score matrix) split 1024 rows per core;
K and V are projected per-shard and exchanged with AllGathers.

Schedule (per core):
- A 1-byte dummy AllGather fires immediately so the NRT first-collective
  rendezvous barrier (~27us) overlaps the phase-1 lead-in instead of
  serializing in front of the K exchange.
- Phase 1: x streams in as eight 128-row blocks over four DMA queues; x^T is
  built with PE transposes. K^T halves are projected first and their
  AllGathers triggered as soon as each half hits DRAM. Q^T follows (it gates
  the score stream), then V is projected directly in natural layout (the bias
  is added with a contraction-1 ones-matmul so ScalarE stays free) and its
  two AllGather halves fire.
- Phase 2 is an exp-throughput-bound stream: score units of 3 key-chunks
  ([128,3,512] f32 PSUM, double-buffered; 3 banks each) alternate between the
  two 512-query groups; each unit is one wide EXP on ScalarE (the only ACT
  work in the window). Softmax denominators accumulate on DVE as bf16
  tensor_tensor adds into a [128,3,512] accumulator per group. All A@V
  matmuls are deferred until after the score stream (V lands mid-stream;
  deferring avoids in-order PE queue stalls) and accumulate into a
  [128,2,512] PSUM tile. Finals: ones-matmul row-sum, reciprocal, PE
  transposes of the output, 1/l scaling, DMA out.

kernel(**inputs) takes the FULL unsharded inputs and returns the full output.
"""

import math

import numpy as np

import concourse.bacc as bacc
import concourse.mybir as mybir
import concourse.tile as tile
from concourse.bass_utils import run_bass_kernel_spmd
from concourse.masks import make_identity

SEQ, EMBED, HEAD = 8192, 2048, 128
NCORES = 8
P = 128

F32 = mybir.dt.float32
BF16 = mybir.dt.bfloat16
F8 = mybir.dt.float8e4
U8 = mybir.dt.uint8

Id = mybir.ActivationFunctionType.Identity
Exp = mybir.ActivationFunctionType.Exp


def emit(nc, seq=SEQ, embed=EMBED, head=HEAD, ncores=NCORES):
    assert head == P
    s_loc = seq // ncores          # query rows per core (1024)
    e_ch = embed // P              # contraction chunks for projections (16)
    b_ch = s_loc // P              # 128-row blocks in local shard (8)
    n_half = s_loc // 2            # projection matmul free dim (512)
    sq_g = 512                     # phase-2 query group width
    n_g = s_loc // sq_g            # 2 groups
    n_t = seq // P                 # key/value chunks (64)
    scale = 1.0 / math.sqrt(head)

    x = nc.dram_tensor("x", [s_loc, embed], BF16, kind="ExternalInput").ap()
    wq = nc.dram_tensor("wq", [embed, head], BF16, kind="ExternalInput").ap()
    wk = nc.dram_tensor("wk", [embed, head], BF16, kind="ExternalInput").ap()
    wv = nc.dram_tensor("wv", [embed, head], BF16, kind="ExternalInput").ap()
    bq = nc.dram_tensor("bq", [head], F32, kind="ExternalInput").ap()
    bk = nc.dram_tensor("bk", [head], F32, kind="ExternalInput").ap()
    bv = nc.dram_tensor("bv", [head], F32, kind="ExternalInput").ap()
    out = nc.dram_tensor("out", [s_loc, head], F32, kind="ExternalOutput").ap()

    with tile.TileContext(nc) as tc:
        with (
            tc.tile_pool(name="consts", bufs=1) as consts,
            tc.tile_pool(name="persist", bufs=1) as persist,
            tc.tile_pool(name="dram", bufs=1, space="DRAM") as dram,
        ):
            ident = consts.tile([P, P], F32)
            make_identity(nc, ident)
            ident_bf = consts.tile([P, P], BF16)
            nc.vector.tensor_copy(ident_bf[:], ident[:])
            ones_f32 = consts.tile([P, 1], F32)
            nc.vector.memset(ones_f32[:], 1.0)
            ones_col = consts.tile([P, 1], BF16)
            nc.vector.tensor_copy(ones_col[:], ones_f32[:])
            ones_row = consts.tile([1, P], BF16)
            nc.vector.memset(ones_row[:], 1.0)

            # persistent SBUF across the whole kernel
            qt_sb = persist.tile([P, s_loc], BF16)           # Q^T own shard
            kt_sb = persist.tile([P, n_t, P], F8)            # K^T full (fp8)
            v_sb = persist.tile([P, n_t, P], BF16)           # V natural full

            # K/V exchanged in a (1/4, 3/4) split: the first quarter lands
            # early to start the exp stream; the rest follows.
            hsz1 = P * (s_loc // 4)
            hsz2 = P * (3 * s_loc // 4)
            ag_k1_in = dram.tile([hsz1], F8)
            ag_k2_in = dram.tile([hsz2], F8)
            ag_v1_in = dram.tile([hsz1], BF16)
            ag_v2_in = dram.tile([hsz2], BF16)
            ag_k1_out = dram.tile([ncores * hsz1], F8, addr_space="Shared")
            ag_k2_out = dram.tile([ncores * hsz2], F8, addr_space="Shared")
            ag_v1_out = dram.tile([ncores * hsz1], BF16, addr_space="Shared")
            ag_v2_out = dram.tile([ncores * hsz2], BF16, addr_space="Shared")

            # ---------------- Phase 1: project own shard ----------------
            with (
                tc.tile_pool(name="p1", bufs=1) as p1,
                tc.tile_pool(name="trps", bufs=3, space="PSUM") as trps,
                tc.tile_pool(name="projps", bufs=2, space="PSUM") as projps,
            ):
                # x natural rows in, spread over four queues.
                x_b = x.rearrange("(b p) e -> b p e", p=P)
                x_sb = p1.tile([P, b_ch, embed], BF16)
                x_eng = [nc.sync, nc.gpsimd, nc.scalar]
                for b in range(b_ch):
                    x_eng[b % 3].dma_start(x_sb[:, b, :], x_b[b])

                bq_sb = p1.tile([P, 1], F32)
                bk_sb = p1.tile([P, 1], F32)
                bv_sb = p1.tile([1, P], F32)
                nc.scalar.dma_start(bq_sb[:], bq.unsqueeze(1))
                nc.scalar.dma_start(bk_sb[:], bk.unsqueeze(1))
                nc.scalar.dma_start(bv_sb[:], bv.unsqueeze(0))

                wq_sb = p1.tile([P, e_ch, head], BF16)
                wk_sb = p1.tile([P, e_ch, head], BF16)
                wv_sb = p1.tile([P, e_ch, head], BF16)
                nc.scalar.dma_start(
                    wk_sb[:], wk.rearrange("(c p) h -> p c h", p=P))
                nc.sync.dma_start(
                    wq_sb[:], wq.rearrange("(c p) h -> p c h", p=P))
                xt = p1.tile([P, e_ch, s_loc], BF16)
                kt_loc = p1.tile([P, s_loc], F8)
                v_nat = p1.tile([P, b_ch, head], BF16)

                def transpose_blocks(b0, b1):
                    for b in range(b0, b1):
                        for eq in range(0, e_ch, 4):
                            tr = trps.tile([P, 4, P], BF16, tag="tr")
                            for j in range(4):
                                e = eq + j
                                nc.tensor.transpose(
                                    tr[:, j, :],
                                    x_sb[:, b, e * P:(e + 1) * P],
                                    ident_bf[:])
                            dst = xt[:, eq:eq + 4, b * P:(b + 1) * P]
                            if (b + eq // 4) % 2 == 0:
                                nc.vector.tensor_copy(dst, tr[:])
                            else:
                                nc.scalar.copy(dst, tr[:])

                def project(w_sb, b_sb, dst, c0, c1, tag="pps"):
                    hsl = slice(c0, c1)
                    ps = projps.tile([P, n_half], F32, tag=tag)
                    for e in range(e_ch):
                        nc.tensor.matmul(
                            ps[:, 0:c1 - c0], w_sb[:, e, :], xt[:, e, hsl],
                            start=(e == 0), stop=(e == e_ch - 1))
                    nc.scalar.activation(dst[:, hsl], ps[:, 0:c1 - c0], Id,
                                         bias=b_sb[:, 0:1])

                def fire_ag(ag_in, ag_out):
                    nc.gpsimd.collective_compute(
                        "AllGather", mybir.AluOpType.bypass,
                        replica_groups=[list(range(ncores))],
                        ins=[ag_in.opt()], outs=[ag_out.opt()])

                # K first: quarter, then the remaining three quarters.
                q_w = s_loc // 4
                transpose_blocks(0, 2)
                project(wk_sb, bk_sb, kt_loc, 0, q_w)
                nc.sync.dma_start(
                    ag_k1_in.rearrange("(p s) -> p s", p=P),
                    kt_loc[:, 0:q_w])
                fire_ag(ag_k1_in, ag_k1_out)
                transpose_blocks(2, 4)
                project(wk_sb, bk_sb, kt_loc, q_w, n_half)
                transpose_blocks(4, 8)
                project(wk_sb, bk_sb, kt_loc, n_half, s_loc)
                nc.sync.dma_start(
                    ag_k2_in.rearrange("(p s) -> p s", p=P),
                    kt_loc[:, q_w:s_loc])
                fire_ag(ag_k2_in, ag_k2_out)

                # wv DMA enqueued only now: its long descriptor-enqueue must
                # not block the gpsimd queue ahead of the K triggers.
                nc.gpsimd.dma_start(
                    wv_sb[:], wv.rearrange("(c p) h -> p c h", p=P))
                bv_bf = p1.tile([1, P], BF16)
                nc.scalar.copy(bv_bf[:], bv_sb[:])

                # Q next: it gates the score stream.
                project(wq_sb, bq_sb, qt_sb, 0, n_half)
                project(wq_sb, bq_sb, qt_sb, n_half, s_loc)

                # V directly in natural layout: per 128-row block, 16
                # accumulating matmuls (stationary = x^T e-chunk) plus a
                # contraction-1 ones-matmul that broadcasts the bias.
                def v_blocks(b0, b1):
                    for b in range(b0, b1):
                        vps = projps.tile([P, head], F32, tag="vps")
                        for e in range(e_ch):
                            nc.tensor.matmul(
                                vps[:], xt[:, e, b * P:(b + 1) * P],
                                wv_sb[:, e, :],
                                start=(e == 0), stop=False)
                        nc.tensor.matmul(
                            vps[:], ones_row[:], bv_bf[:],
                            start=False, stop=True)
                        nc.vector.tensor_copy(v_nat[:, b, :], vps[:])

                v_blocks(0, 2)
                nc.sync.dma_start(
                    ag_v1_in.rearrange("(b p h) -> p b h", p=P, h=head),
                    v_nat[:, 0:2, :])
                fire_ag(ag_v1_in, ag_v1_out)
                v_blocks(2, b_ch)
                nc.sync.dma_start(
                    ag_v2_in.rearrange("(b p h) -> p b h", p=P, h=head),
                    v_nat[:, 2:b_ch, :])
                fire_ag(ag_v2_in, ag_v2_out)

            # unpack gathered K^T / V into SBUF. Slot s in kt_sb and v_sb
            # refer to the same original chunk: wave-1 slots r*2+j (j<2),
            # wave-2 slots 16 + r*6 + j.
            q_ch = 2
            r_ch = 6
            n_w1 = ncores * q_ch
            for r in range(ncores):
                eng = nc.sync if r < 4 else nc.gpsimd
                eng.dma_start(
                    kt_sb[:, r * q_ch:(r + 1) * q_ch, :],
                    ag_k1_out[r * hsz1:(r + 1) * hsz1].rearrange(
                        "(p b t) -> p b t", p=P, b=q_ch, t=P))
            for r in range(ncores):
                nc.gpsimd.dma_start(
                    kt_sb[:, n_w1 + r * r_ch:n_w1 + (r + 1) * r_ch, :],
                    ag_k2_out[r * hsz2:(r + 1) * hsz2].rearrange(
                        "(p b t) -> p b t", p=P, b=r_ch, t=P))
            for r in range(ncores):
                nc.gpsimd.dma_start(
                    v_sb[:, r * q_ch:(r + 1) * q_ch, :],
                    ag_v1_out[r * hsz1:(r + 1) * hsz1].rearrange(
                        "(b p h) -> p b h", p=P, h=head))
            for r in range(ncores):
                nc.gpsimd.dma_start(
                    v_sb[:, n_w1 + r * r_ch:n_w1 + (r + 1) * r_ch, :],
                    ag_v2_out[r * hsz2:(r + 1) * hsz2].rearrange(
                        "(b p h) -> p b h", p=P, h=head))

            # ---------------- Phase 2: attention ----------------
            # Score/exp stream first (ACT-bound), AV matmuls deferred, finals
            # last. Units of 3 chunks; 64 = 21*3 + 1.
            units = [(u * 3, 3) for u in range(21)] + [(63, 1)]
            with (
                tc.tile_pool(name="p2", bufs=1) as p2,
                tc.tile_pool(name="p2s", bufs=2) as p2s,
                tc.tile_pool(name="stps", bufs=2, space="PSUM") as stps,
                tc.tile_pool(name="avps", bufs=1, space="PSUM") as avps,
            ):
                pt = [p2.tile([P, n_t, sq_g], BF16, name=f"pt{g}")
                      for g in range(n_g)]
                acc = [p2.tile([P, 3, sq_g], BF16, name=f"acc{g}")
                       for g in range(n_g)]
                avt = avps.tile([P, n_g, sq_g], F32)

                # A@V jobs interleave into the score stream's PE slack (each
                # exp takes ~1.6us vs ~0.65us of score matmuls): slots 0-39
                # (both groups) ride unit-pairs 8..17; the rest ride the
                # group-serial tail blocks so each group's AV finishes with
                # its exps and finals(g0) overlaps g1's last exps.
                av_jobs = [(s, g) for s in range(40) for g in range(n_g)]
                av_jobs += [(s, 0) for s in range(40, n_t)]
                av_jobs += [(s, 1) for s in range(40, n_t)]

                def av_emit(n):
                    for _ in range(n):
                        if not av_jobs:
                            return
                        s, g = av_jobs.pop(0)
                        nc.tensor.matmul(
                            avt[:, g, :], v_sb[:, s, :], pt[g][:, s, :],
                            start=(s == 0), stop=(s == n_t - 1),
                            skip_group_check=True)

                def unit(g, c0, w):
                    qg = qt_sb[:, g * sq_g:(g + 1) * sq_g]
                    st = stps.tile([P, 3, sq_g], F32, tag="st")
                    for k in range(w):
                        nc.tensor.matmul(
                            st[:, k, :], kt_sb[:, c0 + k, :], qg,
                            start=True, stop=True, skip_group_check=True)
                    nc.scalar.activation(
                        pt[g][:, c0:c0 + w, :], st[:, 0:w, :], Exp,
                        scale=scale)
                    if c0 == 3:
                        nc.vector.tensor_tensor(
                            acc[g][:], pt[g][:, 0:3, :],
                            pt[g][:, 3:6, :], mybir.AluOpType.add)
                    elif c0 > 3:
                        nc.vector.tensor_tensor(
                            acc[g][:, 0:w, :], acc[g][:, 0:w, :],
                            pt[g][:, c0:c0 + w, :],
                            mybir.AluOpType.add)

                def finals(g):
                    nc.vector.tensor_tensor(
                        acc[g][:, 0:1, :], acc[g][:, 0:1, :],
                        acc[g][:, 1:2, :], mybir.AluOpType.add)
                    nc.vector.tensor_tensor(
                        acc[g][:, 0:1, :], acc[g][:, 0:1, :],
                        acc[g][:, 2:3, :], mybir.AluOpType.add)
                    l_ps = stps.tile([1, sq_g], F32, tag="st", name="l_ps")
                    nc.tensor.matmul(
                        l_ps[:], ones_col[:], acc[g][:, 0, :],
                        start=True, stop=True, skip_group_check=True)
                    l_sb = p2s.tile([1, sq_g], F32, tag="lsb")
                    nc.vector.tensor_copy(l_sb[:], l_ps[:])
                    ot_sb = p2s.tile([P, sq_g], F32, tag="otsb")
                    nc.vector.tensor_copy(ot_sb[:], avt[:, g, :])
                    for j in range(sq_g // P):
                        lc_ps = stps.tile([P, 1], F32, tag="st", name="lc_ps")
                        nc.tensor.transpose(
                            lc_ps[:], l_sb[0:1, j * P:(j + 1) * P],
                            ident[0:1, 0:1])
                        r_col = p2s.tile([P, 1], F32, tag="rcol")
                        nc.vector.reciprocal(r_col[:], lc_ps[:])
                        o_tr = stps.tile([P, P], F32, tag="st", name="o_tr")
                        nc.tensor.transpose(
                            o_tr[:], ot_sb[:, j * P:(j + 1) * P],
                            ident[:])
                        o_sb = p2s.tile([P, head], F32, tag="osb")
                        nc.vector.tensor_scalar_mul(
                            o_sb[:], o_tr[:], r_col[:, 0:1])
                        row0 = g * sq_g + j * P
                        nc.sync.dma_start(out[row0:row0 + P, :], o_sb[:])

                for ui, (c0, w) in enumerate(units[:18]):
                    for g in range(n_g):
                        unit(g, c0, w)
                    if ui >= 8:
                        av_emit(8)
                for c0, w in units[18:]:
                    unit(0, c0, w)
                    av_emit(6)
                finals(0)
                for c0, w in units[18:]:
                    unit(1, c0, w)
                    av_emit(6)
                finals(1)
    nc.compile()
    return nc


_CACHE = {}


def _get_nc():
    if "nc" not in _CACHE:
        nc = bacc.Bacc("TRN2", target_bir_lowering=False, debug=False,
                       num_devices=NCORES)
        _CACHE["nc"] = emit(nc)
    return _CACHE["nc"]


def make_in_maps(x, Wq, bq, Wk, bk, Wv, bv):
    import ml_dtypes
    bf = ml_dtypes.bfloat16
    x = np.ascontiguousarray(np.asarray(x, dtype=np.float32).astype(bf))
    Wq = np.ascontiguousarray(np.asarray(Wq, dtype=np.float32).astype(bf))
    Wk = np.ascontiguousarray(np.asarray(Wk, dtype=np.float32).astype(bf))
    Wv = np.ascontiguousarray(np.asarray(Wv, dtype=np.float32).astype(bf))
    bq = np.ascontiguousarray(np.asarray(bq, dtype=np.float32))
    bk = np.ascontiguousarray(np.asarray(bk, dtype=np.float32))
    bv = np.ascontiguousarray(np.asarray(bv, dtype=np.float32))
    s_loc = SEQ // NCORES
    return [
        {
            "x": np.ascontiguousarray(x[c * s_loc:(c + 1) * s_loc]),
            "wq": Wq, "wk": Wk, "wv": Wv,
            "bq": bq, "bk": bk, "bv": bv,
        }
        for c in range(NCORES)
    ]


def kernel(x, Wq, bq, Wk, bk, Wv, bv):
    in_maps = make_in_maps(x, Wq, bq, Wk, bk, Wv, bv)
    res = run_bass_kernel_spmd(_get_nc(), in_maps,
                               core_ids=list(range(NCORES)))
    return np.concatenate(
        [res.results[c]["out"] for c in range(NCORES)], axis=0)
